# revision 1
# baseline (speedup 1.0000x reference)
"""Trainium2 Bass kernel for nn_Net_21818433863930 (interaction-network GNN).

Contract: kernel(**inputs) takes FULL unsharded fp32 inputs (z: (8192, 8, 16) plus
all MLP weights), shards batch across 8 NeuronCores (pure data parallel), runs a
Bass/Tile kernel per core, and returns the FULL (8192, 8, 32) fp32 output.

Architecture notes (per core, n_loc=1024 samples, C=8192 object-cols, feature-major):
  - everything is computed feature-major: SBUF tiles are (features, columns)
  - pair grid (sample, i, j) processed as 16 chunks x 512 object-cols x 9 j-slabs
    (slabs 0..7 = partner j, slab 8 = diagonal pairs with NEGATED aggregation
    weights, which implements the (1 - eye) mask exactly by cancellation)
  - rel-MLP layer 3 commutes with the attention-weighted sum over j, so the
    j-aggregation IS a PSUM-accumulated matmul with lhsT=[rw2; rb2]
  - dist = sqrt(r_i + r_j - 2*G + eps) assembled by PSUM accumulation on the PE
"""
import numpy as np

N, O, CL = 8192, 8, 32
D_IN = 16
EPS = 1e-12
N_CORES = 8
TC = 512  # columns per chunk

_F32 = None  # set lazily (mybir.dt.float32)


# ---------------------------------------------------------------- host packing
def pack_weights(inp: dict) -> dict:
    """Pack reference weights into lhsT/bias arrays the kernel consumes.
    All matmul weights are (K, M) = (in_features, out_features) fp32."""
    f32 = np.float32
    w = {}
    w["wenc"] = np.ascontiguousarray(inp["Wenc"], f32)            # (16, 32)
    w["benc"] = np.ascontiguousarray(inp["benc"].reshape(32, 1), f32)

    for c in range(3):
        rw0, aw0 = inp["rw0"][c], inp["aw0"][c]                   # (65, 64)
        w[f"w1pa_{c}"] = np.concatenate([rw0[0:32], aw0[0:32]], axis=1).astype(f32)
        w[f"w1pb_{c}"] = np.concatenate([rw0[32:64], aw0[32:64]], axis=1).astype(f32)
        w[f"w1pc_{c}"] = np.concatenate([rw0[64:65], aw0[64:65]], axis=1).astype(f32)
        w[f"b1_{c}"] = np.concatenate([inp["rb0"][c], inp["ab0"][c]]).reshape(128, 1).astype(f32)

        # a1 rows: 0:64 rel1, 64:128 att1.
        # a2 rows: 0:32 m(rel2), 32 ones, 33:64 zero, 64:96 t(att2)
        # (everything an elementwise op touches is base-partition aligned)
        w2p = np.zeros((128, 96), f32)
        w2p[0:64, 0:32] = inp["rw1"][c]                            # rel branch -> m
        w2p[64:128, 64:96] = inp["aw1"][c]                         # att branch -> t
        w[f"w2p_{c}"] = w2p
        b2 = np.zeros((96, 1), f32)
        b2[0:32, 0] = inp["rb1"][c]; b2[32, 0] = 1.0; b2[64:96, 0] = inp["ab1"][c]
        w[f"b2_{c}"] = b2

        w3a = np.zeros((96, 33), f32)
        w3a[64:96, :] = np.tile(inp["aw2"][c].reshape(32, 1), (1, 33))
        w[f"w3a_{c}"] = w3a
        w[f"ab2r_{c}"] = np.full((33, 1), float(inp["ab2"][c][0]), f32)

        w3r = np.zeros((33, 32), f32)
        w3r[0:32] = inp["rw2"][c]; w3r[32] = inp["rb2"][c]
        w[f"w3r_{c}"] = w3r

    def bd(ms):  # block-diag of three (32, 32)
        out = np.zeros((96, 96), f32)
        for c in range(3):
            out[32*c:32*c+32, 32*c:32*c+32] = ms[c]
        return out

    w["wself0s"] = np.concatenate([inp["sw0"][c] for c in range(3)], axis=1).astype(f32)  # (32, 96)
    w["sb0s"] = np.concatenate([inp["sb0"][c] for c in range(3)]).reshape(96, 1).astype(f32)
    w["wself1bd"] = bd([inp["sw1"][c] for c in range(3)])
    w["dynb"] = np.concatenate([inp["sb1"][c] for c in range(3)]).reshape(96, 1).astype(f32)

    for k, (wn, bn) in enumerate([("fw0", "fb0"), ("fw1", "fb1"), ("fw2", "fb2")]):
        w[f"waff{k}bd"] = bd([inp[wn][c] for c in range(3)])
        w[f"fb{k}s"] = np.concatenate([inp[bn][c] for c in range(3)]).reshape(96, 1).astype(f32)

    w["wow0abd"] = bd([inp["ow0"][c][0:32] for c in range(3)])
    w["wow0ss"] = np.concatenate([inp["ow0"][c][32:64] for c in range(3)], axis=1).astype(f32)  # (32, 96)
    w["ob0s"] = np.concatenate([inp["ob0"][c] for c in range(3)]).reshape(96, 1).astype(f32)
    w["wow1bd"] = bd([inp["ow1"][c] for c in range(3)])

    ob1cat = np.concatenate([inp["ob1"][c] for c in range(3)])     # (96,)
    w["wagg1"] = np.ascontiguousarray(inp["Wagg1"], f32)           # (96, 32)
    w["bagg1p"] = (inp["Wagg1"].T @ ob1cat + inp["bagg1"]).reshape(32, 1).astype(f32)
    w["wagg2"] = np.ascontiguousarray(inp["Wagg2"], f32)           # (32, 32)
    w["bagg2"] = np.ascontiguousarray(inp["bagg2"].reshape(32, 1), f32)

    w["ones32"] = np.ones((32, 1), f32)
    w["negtwo32"] = np.full((32, 1), -2.0, f32)
    for c in range(3):
        w3rp = np.zeros((33, 96), f32)
        w3rp[:, 32*c:32*c+32] = w[f"w3r_{c}"]
        w[f"w3rp_{c}"] = w3rp; w[f"w3rn_{c}"] = -w3rp
        del w[f"w3r_{c}"]
    # d2 = -2*sum(prod) + r_j + r_i reduction weights
    dw33 = np.full((33, 1), -2.0, f32); dw33[32] = 1.0
    w["dw33"] = dw33
    w["one1"] = np.ones((1, 1), f32)
    onep32 = np.zeros((33, 1), f32); onep32[32] = 1.0
    w["onep32"] = onep32  # lhsT slice [32:33] -> 1.0 at base partition 32
    w["epsb"] = np.full((1, 1), EPS, f32)
    return w


WEIGHT_SPECS = None  # filled by build_nc


# ---------------------------------------------------------------- device build
def build_nc(n_loc: int, repeat: int = 1, am_engine: str = 'gpsimd', a1_engine: str = 'act', bufs_sbuf: int = 3):
    """Build + compile the per-core Bass program. n_loc = samples per core."""
    import concourse.bass as bass
    import concourse.bacc as bacc
    import concourse.tile as tile
    import concourse.mybir as mybir
    from contextlib import ExitStack

    f32 = mybir.dt.float32
    f32r = mybir.dt.float16  # compute dtype: full PE rate, 2x DVE modes
    AF = mybir.ActivationFunctionType
    OP = mybir.AluOpType

    def mm(out, lhsT, rhs, **kw):
        nc.tensor.matmul(out, lhsT, rhs, **kw)

    C = n_loc * O
    n_chunks = C // TC
    assert C % TC == 0

    nc = bacc.Bacc("TRN2", target_bir_lowering=False, debug=False,
                   enable_asserts=False)

    zT = nc.dram_tensor("zT", (D_IN, C), f32r, kind="ExternalInput").ap()
    outT = nc.dram_tensor("outT", (CL, C), f32, kind="ExternalOutput").ap()

    wspecs = {
        "wenc": (16, 32), "benc": (32, 1),
        "wself0s": (32, 96), "sb0s": (96, 1), "wself1bd": (96, 96), "dynb": (96, 1),
        "waff0bd": (96, 96), "fb0s": (96, 1), "waff1bd": (96, 96), "fb1s": (96, 1),
        "waff2bd": (96, 96), "fb2s": (96, 1),
        "wow0abd": (96, 96), "wow0ss": (32, 96), "ob0s": (96, 1), "wow1bd": (96, 96),
        "wagg1": (96, 32), "bagg1p": (32, 1), "wagg2": (32, 32), "bagg2": (32, 1),
        "ones32": (32, 1), "negtwo32": (32, 1), "one1": (1, 1), "onep32": (33, 1), "epsb": (1, 1),
    }
    for c in range(3):
        wspecs[f"w1pa_{c}"] = (32, 128); wspecs[f"w1pb_{c}"] = (32, 128); wspecs[f"w1pc_{c}"] = (1, 128); wspecs[f"b1_{c}"] = (128, 1)
        wspecs[f"w2p_{c}"] = (128, 96); wspecs[f"b2_{c}"] = (96, 1)
        wspecs[f"w3a_{c}"] = (96, 33); wspecs[f"ab2r_{c}"] = (33, 1)
    for c in range(3):
        wspecs[f"w3rp_{c}"] = (33, 96); wspecs[f"w3rn_{c}"] = (33, 96)
    wspecs["epsrow"] = (1, C)
    wspecs["dw33"] = (33, 1)
    global WEIGHT_SPECS
    WEIGHT_SPECS = wspecs

    MMW = set(['wenc', 'wself0s', 'wself1bd', 'waff0bd', 'waff1bd', 'waff2bd', 'wow0abd', 'wow0ss', 'wow1bd', 'wagg1', 'wagg2', 'ones32', 'negtwo32', 'one1', 'dw33', 'epsrow', 'w1pa_0', 'w1pa_1', 'w1pa_2', 'w1pb_0', 'w1pb_1', 'w1pb_2', 'w1pc_0', 'w1pc_1', 'w1pc_2', 'w2p_0', 'w2p_1', 'w2p_2', 'w3a_0', 'w3a_1', 'w3a_2', 'w3rp_0', 'w3rp_1', 'w3rp_2', 'w3rn_0', 'w3rn_1', 'w3rn_2'])
    wdram = {k: nc.dram_tensor(k, shp, f32r if k in MMW else f32,
                               kind="ExternalInput").ap()
             for k, shp in wspecs.items()}

    with tile.TileContext(nc) as tc:
        with ExitStack() as ctx:
            # --- persistent weight tiles
            wpool = ctx.enter_context(tc.tile_pool(name="w", bufs=1))
            W = {}
            for k, shp in wspecs.items():
                W[k] = wpool.tile(list(shp), f32r if k in MMW else f32,
                                  tag=k, name=f"w_{k}")
                nc.sync.dma_start(W[k][:], wdram[k][:])

            # persistent tensors
            pers = ctx.enter_context(tc.tile_pool(name="pers", bufs=1))
            stf = pers.tile([32, C], f32r, tag="stf", name="stf")
            aux = pers.tile([O, C], f32r, tag="aux", name="aux")  # rows 0:8 = dist_j

            # --- pools
            p_zt = ctx.enter_context(tc.tile_pool(name="zt", bufs=2))
            p_sq = ctx.enter_context(tc.tile_pool(name="sq", bufs=2))
            p_hself = ctx.enter_context(tc.tile_pool(name="hself", bufs=2))
            p_x = ctx.enter_context(tc.tile_pool(name="x", bufs=3))
            p_prod = ctx.enter_context(tc.tile_pool(name="prod", bufs=2))
            p_dt = ctx.enter_context(tc.tile_pool(name="dt", bufs=2))
            p_a1 = ctx.enter_context(tc.tile_pool(name="a1", bufs=3))
            p_a2 = ctx.enter_context(tc.tile_pool(name="a2", bufs=3))
            p_al = ctx.enter_context(tc.tile_pool(name="al", bufs=3))
            p_am = ctx.enter_context(tc.tile_pool(name="am", bufs=3))
            p_ep = ctx.enter_context(tc.tile_pool(name="ep", bufs=4))
            p_out = ctx.enter_context(tc.tile_pool(name="outs", bufs=2))

            ps_A = ctx.enter_context(tc.tile_pool(name="psA", bufs=2, space="PSUM"))
            ps_B = ctx.enter_context(tc.tile_pool(name="psB", bufs=2, space="PSUM"))
            ps_S = ctx.enter_context(tc.tile_pool(name="psS", bufs=2, space="PSUM"))
            ps_D = ctx.enter_context(tc.tile_pool(name="psD", bufs=2, space="PSUM"))

            for _rep in range(repeat):
              # ============ PASS 0: encoder + r + dist (sqrt table resident)
              for cc in range(n_chunks):
                cs = cc * TC
                s0 = cs // O          # first sample of chunk
                ns = TC // O          # samples per chunk
                zt = p_zt.tile([D_IN, TC], f32r, tag="zt", name="zt")
                nc.sync.dma_start(zt[:], zT[:, cs:cs+TC])
                ps_enc = ps_S.tile([33, TC], f32, tag="ps_s", name="ps_enc")
                mm(ps_enc[0:32, :], W["wenc"][:], zt[:], start=True, stop=True)
                nc.vector.tensor_scalar(stf[0:32, cs:cs+TC], ps_enc[0:32, :],
                                        W["benc"][:], None, op0=OP.add)
                st_soi = stf[0:32, :].rearrange("p (s i) -> p s i", i=O)
                for j in range(O):
                    diff = p_sq.tile([32, TC], f32r, tag="diff", name="diff")
                    nc.vector.tensor_tensor(
                        diff[:].rearrange("p (s i) -> p s i", i=O),
                        st_soi[:, s0:s0+ns, :],
                        st_soi[:, s0:s0+ns, j:j+1].broadcast_to((32, ns, O)),
                        op=OP.subtract)
                    dsq = p_prod.tile([32, TC], f32r, tag="dsq", name="dsq")
                    nc.vector.tensor_tensor(dsq[:], diff[:], diff[:], op=OP.mult)
                    ps_d2 = ps_S.tile([33, TC], f32, tag="ps_s", name="ps_d2")
                    mm(ps_d2[0:1, :], W["ones32"][:], dsq[:], start=True, stop=True)
                    dtmp = p_dt.tile([1, TC], f32r, tag="dtmp", name="dtmp")
                    nc.scalar.activation(dtmp[:], ps_d2[0:1, :], AF.Sqrt, bias=W["epsb"][:])
                    nc.gpsimd.dma_start(aux[j:1+j, cs:cs+TC], dtmp[:])

              # ============ MAIN PASS (sigmoid table resident)
              for cc in range(n_chunks):
                cs = cc * TC
                s0 = cs // O
                ns = TC // O
                st_soi = stf[0:32, :].rearrange("p (s i) -> p s i", i=O)
                hself = p_hself.tile([96, TC], f32r, tag="hself", name="hself")
                ps_h = ps_D.tile([96, TC], f32, tag="ps_d", name="ps_h")
                mm(ps_h[:], W["wself0s"][:], stf[0:32, cs:cs+TC], start=True, stop=True)
                nc.vector.tensor_scalar(hself[:], ps_h[:], W["sb0s"][:], 0.0,
                                        op0=OP.add, op1=OP.max)

                dyn_acc = ps_D.tile([96, TC], f32, tag="ps_d", name="dyn_acc")

                for j in range(O + 1):
                    if j < O:
                        xjt = p_x.tile([32, TC], f32r, tag="x", name="xjt")
                        nc.gpsimd.tensor_copy(
                            xjt[:].rearrange("p (s i) -> p s i", i=O),
                            st_soi[:, s0:s0+ns, j:j+1].broadcast_to((32, ns, O)))
                        xb = xjt[:, :]
                        xd = p_dt.tile([1, TC], f32r, tag="xd", name="xd")
                        nc.scalar.dma_start(xd[:], aux[j:1+j, cs:cs+TC])
                        xdist = xd[:, :]
                    else:  # diagonal slab: partner = self, dist = sqrt(eps)
                        xb = stf[0:32, cs:cs+TC]
                        xdist = W["epsrow"][0:1, cs:cs+TC]

                    for c in range(3):
                        psA = ps_A.tile([128, TC], f32, tag="psA", name="psA")
                        mm(psA[:], W[f"w1pa_{c}"][:], stf[0:32, cs:cs+TC],
                           start=True, stop=False)
                        mm(psA[:], W[f"w1pb_{c}"][:], xb, start=False, stop=False)
                        mm(psA[:], W[f"w1pc_{c}"][:], xdist, start=False, stop=True)
                        a1 = p_a1.tile([128, TC], f32r, tag="a1", name="a1")
                        nc.scalar.activation(a1[:], psA[:], AF.Relu, bias=W[f"b1_{c}"][:])
                        psB = ps_B.tile([96, TC], f32, tag="psB", name="psB")
                        mm(psB[:], W[f"w2p_{c}"][:], a1[:], start=True, stop=True)
                        a2 = p_a2.tile([96, TC], f32r, tag="a2", name="a2")
                        nc.vector.tensor_scalar(a2[:], psB[:], W[f"b2_{c}"][:], 0.0,
                                                op0=OP.add, op1=OP.max)
                        psC = ps_S.tile([33, TC], f32, tag="ps_s", name="psC")
                        mm(psC[:], W[f"w3a_{c}"][64:96, :], a2[64:96, :], start=True, stop=True)
                        alr = p_al.tile([33, TC], f32r, tag="al", name="alr")
                        nc.scalar.activation(alr[:], psC[:], AF.Sigmoid, bias=W[f"ab2r_{c}"][:])
                        am = p_am.tile([33, TC], f32r, tag="am", name="am")
                        nc.vector.tensor_tensor(am[:], a2[0:33, :], alr[:], op=OP.mult)
                        wkey = f"w3rp_{c}" if j < O else f"w3rn_{c}"
                        mm(dyn_acc[:], W[wkey][:], am[:],
                           start=(j == 0 and c == 0), stop=False,
                           skip_group_check=True)

                # self-dynamics into the same accumulator, then evacuate
                mm(dyn_acc[:], W["wself1bd"][:], hself[:],
                   start=False, stop=True, skip_group_check=True)
                dyn = p_ep.tile([96, TC], f32r, tag="ep", name="dyn")
                nc.vector.tensor_scalar(dyn[:], dyn_acc[:], W["dynb"][:], None, op0=OP.add)

                # ---- affector + out + agg chains
                cur = dyn
                for k in range(3):
                    psE = ps_D.tile([96, TC], f32, tag="ps_d", name="psE")
                    mm(psE[:], W[f"waff{k}bd"][:], cur[:], start=True, stop=True)
                    nxt = p_ep.tile([96, TC], f32r, tag="ep", name="nxt")
                    if k < 2:
                        nc.vector.tensor_scalar(nxt[:], psE[:], W[f"fb{k}s"][:], 0.0,
                                                op0=OP.add, op1=OP.max)
                    else:
                        nc.vector.tensor_scalar(nxt[:], psE[:], W[f"fb{k}s"][:], None, op0=OP.add)
                    cur = nxt
                psO = ps_D.tile([96, TC], f32, tag="ps_d", name="psO")
                mm(psO[:], W["wow0abd"][:], cur[:], start=True, stop=False)
                mm(psO[:], W["wow0ss"][:], stf[0:32, cs:cs+TC], start=False, stop=True)
                o0 = p_ep.tile([96, TC], f32r, tag="ep", name="o0")
                nc.vector.tensor_scalar(o0[:], psO[:], W["ob0s"][:], 0.0,
                                        op0=OP.add, op1=OP.max)
                psO1 = ps_D.tile([96, TC], f32, tag="ps_d", name="psO1")
                mm(psO1[:], W["wow1bd"][:], o0[:], start=True, stop=True)
                ccat = p_ep.tile([96, TC], f32r, tag="ep", name="ccat")
                nc.vector.tensor_scalar(ccat[:], psO1[:], 0.0, None, op0=OP.add)
                psG = ps_S.tile([33, TC], f32, tag="ps_s", name="psG")
                mm(psG[0:32, :], W["wagg1"][:], ccat[:], start=True, stop=True)
                h = p_ep.tile([32, TC], f32r, tag="ep", name="h")
                nc.vector.tensor_scalar(h[:], psG[0:32, :], W["bagg1p"][:], 0.0,
                                        op0=OP.add, op1=OP.max)
                psG2 = ps_S.tile([33, TC], f32, tag="ps_s", name="psG2")
                mm(psG2[0:32, :], W["wagg2"][:], h[:], start=True, stop=True)
                ot = p_out.tile([32, TC], f32, tag="ot", name="ot")
                nc.vector.tensor_scalar(ot[:], psG2[0:32, :], W["bagg2"][:], None, op0=OP.add)
                nc.sync.dma_start(outT[:, cs:cs+TC], ot[:])

    nc.compile()
    return nc


# ---------------------------------------------------------------- host runner
_CACHE = {}


def _make_runner(nc, n_cores=N_CORES):
    import jax
    import numpy as _np
    import concourse.mybir as mybir
    from concourse import bass2jax
    from jax.sharding import Mesh, PartitionSpec
    from jax.experimental.shard_map import shard_map

    bass2jax.install_neuronx_cc_hook()
    partition_name = nc.partition_id_tensor.name if nc.partition_id_tensor else None
    in_names, out_names, out_avals, zero_shapes = [], [], [], []
    for alloc in nc.m.functions[0].allocations:
        if not isinstance(alloc, mybir.MemoryLocationSet):
            continue
        name = alloc.memorylocations[0].name
        if alloc.kind == "ExternalInput":
            if name != partition_name:
                in_names.append(name)
        elif alloc.kind == "ExternalOutput":
            out_names.append(name)
            shape = tuple(alloc.tensor_shape)
            dtype = mybir.dt.np(alloc.dtype)
            out_avals.append(jax.core.ShapedArray(shape, dtype))
            zero_shapes.append((shape, dtype))
    n_params = len(in_names)
    n_outs = len(out_avals)
    all_in_names = in_names + out_names + ([partition_name] if partition_name else [])
    donate = tuple(range(n_params, n_params + n_outs))

    def _body(*args):
        operands = list(args)
        if partition_name is not None:
            operands.append(bass2jax.partition_id_tensor())
        outs = bass2jax._bass_exec_p.bind(
            *operands, out_avals=tuple(out_avals), in_names=tuple(all_in_names),
            out_names=tuple(out_names), lowering_input_output_aliases=(),
            sim_require_finite=False, sim_require_nnan=False, nc=nc)
        return tuple(outs)

    devices = jax.devices()[:n_cores]
    mesh = Mesh(_np.asarray(devices), ("core",))
    sharded = jax.jit(
        shard_map(_body, mesh=mesh,
                  in_specs=(PartitionSpec("core"),) * (n_params + n_outs),
                  out_specs=(PartitionSpec("core"),) * n_outs,
                  check_rep=False),
        donate_argnums=donate, keep_unused=True)

    def run(in_maps):
        per_core = [[_np.asarray(m[name]) for name in in_names] for m in in_maps]
        concat_in = [_np.concatenate([per_core[c][i] for c in range(n_cores)], axis=0)
                     for i in range(n_params)]
        concat_zeros = [_np.zeros((n_cores * s[0], *s[1:]), d) for s, d in zero_shapes]
        out_arrs = sharded(*concat_in, *concat_zeros)
        jax.block_until_ready(out_arrs)
        return [
            {name: _np.asarray(out_arrs[i]).reshape(n_cores, *out_avals[i].shape)[c]
             for i, name in enumerate(out_names)}
            for c in range(n_cores)
        ]
    return run


_MMW = ['wenc', 'wself0s', 'wself1bd', 'waff0bd', 'waff1bd', 'waff2bd',
        'wow0abd', 'wow0ss', 'wow1bd', 'wagg1', 'wagg2', 'ones32', 'negtwo32',
        'one1', 'dw33', 'epsrow'] +        [f"w1pa_{c}" for c in range(3)] + [f"w1pb_{c}" for c in range(3)] +        [f"w1pc_{c}" for c in range(3)] + [f"w2p_{c}" for c in range(3)] +        [f"w3a_{c}" for c in range(3)] + [f"w3rp_{c}" for c in range(3)] +        [f"w3rn_{c}" for c in range(3)]


def make_in_maps(inputs: dict, n_loc: int, n_cores: int = N_CORES):
    w = pack_weights(inputs)
    for k in _MMW:
        if k in w:
            w[k] = w[k].astype(np.float16)
    z = np.asarray(inputs["z"], np.float32)
    in_maps = []
    for c in range(n_cores):
        zc = z[c*n_loc:(c+1)*n_loc].reshape(n_loc * O, D_IN)
        m = dict(w)
        m["zT"] = np.ascontiguousarray(zc.T).astype(np.float16)
        m["epsrow"] = np.full((1, n_loc * O), 1e-6, np.float16)
        in_maps.append(m)
    return in_maps


def kernel(**inputs) -> np.ndarray:
    n = inputs["z"].shape[0]
    n_loc = n // N_CORES
    key = ("k", n_loc)
    if key not in _CACHE:
        nc = build_nc(n_loc)
        _CACHE[key] = (nc, _make_runner(nc))
    nc, runner = _CACHE[key]
    res = runner(make_in_maps(inputs, n_loc))
    out = np.concatenate(
        [res[c]["outT"].T.reshape(n_loc, O, CL) for c in range(N_CORES)], axis=0)
    return out



# revision 34
# speedup vs baseline: 4.1913x; 4.1913x over previous
"""Trainium2 Bass kernel for nn_Net_21818433863930 (interaction-network GNN).

Contract: kernel(**inputs) takes FULL unsharded fp32 inputs (z: (8192, 8, 16) plus
all MLP weights), shards batch across 8 NeuronCores (pure data parallel), runs a
Bass/Tile kernel per core, and returns the FULL (8192, 8, 32) fp32 output.

v2 layout (per core, n_loc=1024 samples, C=8192 object-cols, feature-major):
  - SBUF tiles are (features, columns); pair grid processed as 16 chunks x 512
    object-cols x 9 j-slabs (slab 8 = diagonal, aggregated with NEGATED weights
    to realize the (1-eye) mask by cancellation)
  - the three interaction cores' rel/att stages are STACKED: psBm (3x33 rows)
    and psBt (3x32 rows) PSUM tiles collect all cores' hidden layers so the
    relu/sigmoid/mult/aggregate steps run once per j-slab instead of once per
    (j, core) -- block-diagonal lhsT weights do the bookkeeping
  - dist is a 33rd input row of the partner tile (w1pbc = [w1pb; w1pc]); the
    diagonal slab merges both psA matmuls into one (w1d = w1pa + w1pb)
  - pass 0 (sqrt act table) computes encoder + pair distances, writing dist rows
    straight from the Act engine into the persistent aux tile (no DMA roundtrip)
"""
import numpy as np

N, O, CL = 8192, 8, 32
D_IN = 16
EPS = 1e-12
N_CORES = 8
TC = 512  # columns per chunk


# ---------------------------------------------------------------- host packing
def pack_weights(inp: dict) -> dict:
    """Pack reference weights into lhsT/bias arrays the kernel consumes.
    All matmul weights are (K, M) = (in_features, out_features) fp32."""
    f32 = np.float32
    w = {}
    w["wenc"] = np.ascontiguousarray(inp["Wenc"], f32)            # (16, 32)
    w["benc"] = np.ascontiguousarray(inp["benc"].reshape(32, 1), f32)

    for c in range(3):
        rw0, aw0 = inp["rw0"][c], inp["aw0"][c]                   # (65, 64)
        # psA lhsT: self rows (s_i) and partner rows (s_j + dist)
        w[f"w1pa_{c}"] = np.concatenate([rw0[0:32], aw0[0:32]], axis=1).astype(f32)  # (32,128)
        w1pb = np.concatenate([rw0[32:64], aw0[32:64]], axis=1)                      # (32,128)
        w1pc = np.concatenate([rw0[64:65], aw0[64:65]], axis=1)                      # (1,128)
        w[f"w1pbc_{c}"] = np.concatenate([w1pb, w1pc], axis=0).astype(f32)           # (33,128)
        w[f"w1d_{c}"] = (w[f"w1pa_{c}"] + w1pb).astype(f32)                          # (32,128)
        w[f"b1_{c}"] = np.concatenate([inp["rb0"][c], inp["ab0"][c]]).reshape(128, 1).astype(f32)

        # psBm: a1(128) -> rel-hidden2 (32) written at offset 32c
        w2pm = np.zeros((128, 32), f32)
        w2pm[0:64, :] = inp["rw1"][c]
        w[f"w2pm_{c}"] = w2pm
        # psBt: a1(128) -> att-hidden2 (32) at offset 32c
        w2pt = np.zeros((128, 32), f32)
        w2pt[64:128, :] = inp["aw1"][c]
        w[f"w2pt_{c}"] = w2pt

    # biases for the stacked m3 (96,1) / att3 (96,1) evacuations
    b2m = np.zeros((96, 1), f32)
    b2t = np.zeros((96, 1), f32)
    ab2r9 = np.zeros((99, 1), f32)
    for c in range(3):
        b2m[32*c:32*c+32, 0] = inp["rb1"][c]
        b2t[32*c:32*c+32, 0] = inp["ab1"][c]
        ab2r9[96+c, 0] = float(inp["ab2"][c][0])
        ab2r9[32*c:32*c+32, 0] = float(inp["ab2"][c][0])
    w["b2m"] = b2m
    w["b2t"] = b2t
    w["ab2r9"] = ab2r9

    # psC lhsT: att3 (96) -> 99 rows: rows 32c = 32 broadcast copies of core
    # c's logit (for the am product), rows 96:99 = per-core scalar logits
    w3at = np.zeros((96, 99), f32)
    for c in range(3):
        w3at[32*c:32*c+32, 96+c] = inp["aw2"][c].reshape(32)
        w3at[32*c:32*c+32, 32*c:32*c+32] = np.tile(inp["aw2"][c].reshape(32, 1), (1, 32))
    w["w3at"] = w3at

    # aggregation lhsT: am3 (96) -> dyn (96) block-diag rw2; sigma rows -> rb2.
    # w3rb3q is consumed as lhsT at partition base 64 (to match rhs=alr[64:99])
    # so rows 0:32 (the sigma2-broadcast rows of alr) carry zero weights.
    w3rbd = np.zeros((96, 96), f32)
    w3rb3 = np.zeros((35, 96), f32)
    for c in range(3):
        w3rbd[32*c:32*c+32, 32*c:32*c+32] = inp["rw2"][c]
        w3rb3[32+c, 32*c:32*c+32] = inp["rb2"][c]
    w["w3rbd"] = w3rbd
    w["w3rbdn"] = -w3rbd
    w["w3rb3q"] = w3rb3
    w["w3rb3qn"] = -w3rb3

    def bd(ms):  # block-diag of three (32, 32)
        out = np.zeros((96, 96), f32)
        for c in range(3):
            out[32*c:32*c+32, 32*c:32*c+32] = ms[c]
        return out

    w["wself0s"] = np.concatenate([inp["sw0"][c] for c in range(3)], axis=1).astype(f32)  # (32, 96)
    w["sb0s"] = np.concatenate([inp["sb0"][c] for c in range(3)]).reshape(96, 1).astype(f32)
    w["wself1bd"] = bd([inp["sw1"][c] for c in range(3)])
    w["dynb"] = np.concatenate([inp["sb1"][c] for c in range(3)]).reshape(96, 1).astype(f32)

    for k, (wn, bn) in enumerate([("fw0", "fb0"), ("fw1", "fb1"), ("fw2", "fb2")]):
        w[f"waff{k}bd"] = bd([inp[wn][c] for c in range(3)])
        w[f"fb{k}s"] = np.concatenate([inp[bn][c] for c in range(3)]).reshape(96, 1).astype(f32)

    w["wow0abd"] = bd([inp["ow0"][c][0:32] for c in range(3)])
    w["wow0ss"] = np.concatenate([inp["ow0"][c][32:64] for c in range(3)], axis=1).astype(f32)  # (32, 96)
    w["ob0s"] = np.concatenate([inp["ob0"][c] for c in range(3)]).reshape(96, 1).astype(f32)

    ob1cat = np.concatenate([inp["ob1"][c] for c in range(3)])     # (96,)
    # ow1 (per-core, no nonlinearity after) fused into agg1: (ow1_bd @ Wagg1)
    w["wagg1c"] = (bd([inp["ow1"][c] for c in range(3)]).astype(np.float64)
                   @ np.asarray(inp["Wagg1"], np.float64)).astype(f32)  # (96, 32)
    w["bagg1p"] = (inp["Wagg1"].T @ ob1cat + inp["bagg1"]).reshape(32, 1).astype(f32)
    w["wagg2"] = np.ascontiguousarray(inp["Wagg2"], f32)           # (32, 32)
    w["bagg2"] = np.ascontiguousarray(inp["bagg2"].reshape(32, 1), f32)

    w["ones32"] = np.ones((32, 1), f32)
    w["epsb"] = np.full((1, 1), EPS, f32)
    return w


# weights consumed by the PE (matmul lhsT / rhs) -> shipped as fp16
_MMW = (["wenc", "wself0s", "wself1bd", "waff0bd", "waff1bd", "waff2bd",
         "wow0abd", "wow0ss", "wagg1c", "wagg2", "ones32",
         "w3at", "w3rbd", "w3rbdn"]
        + ["w3rb3q", "w3rb3qn"]
        + [f"w1pa_{c}" for c in range(3)] + [f"w1pbc_{c}" for c in range(3)]
        + [f"w1d_{c}" for c in range(3)] + [f"w2pm_{c}" for c in range(3)]
        + [f"w2pt_{c}" for c in range(3)])

_WSPECS = {
    "wenc": (16, 32), "benc": (32, 1),
    "wself0s": (32, 96), "sb0s": (96, 1), "wself1bd": (96, 96), "dynb": (96, 1),
    "waff0bd": (96, 96), "fb0s": (96, 1), "waff1bd": (96, 96), "fb1s": (96, 1),
    "waff2bd": (96, 96), "fb2s": (96, 1),
    "wow0abd": (96, 96), "wow0ss": (32, 96), "ob0s": (96, 1),
    "wagg1c": (96, 32), "bagg1p": (32, 1), "wagg2": (32, 32), "bagg2": (32, 1),
    "ones32": (32, 1), "epsb": (1, 1),
    "b2m": (96, 1), "b2t": (96, 1), "ab2r9": (99, 1),
    "w3at": (96, 99), "w3rbd": (96, 96), "w3rbdn": (96, 96),
    "w3rb3q": (35, 96), "w3rb3qn": (35, 96),
}
# weights consumed as lhsT at a nonzero partition base (must match rhs base)
_PACK_ROWOFF = {"w3rb3q": 64, "w3rb3qn": 64}
for _c in range(3):
    _WSPECS[f"w1pa_{_c}"] = (32, 128)
    _WSPECS[f"w1pbc_{_c}"] = (33, 128)
    _WSPECS[f"w1d_{_c}"] = (32, 128)
    _WSPECS[f"b1_{_c}"] = (128, 1)
    _WSPECS[f"w2pm_{_c}"] = (128, 32)
    _WSPECS[f"w2pt_{_c}"] = (128, 32)

# packed-weight column offsets: one fp16 pack and one fp32 pack, each a single
# (128, ncols) dram tensor -> single DMA (the per-tensor path costs ~565ns of
# DMA queue time per transfer, ~35us of startup for ~60 weights)
_PACK16_OFF, _PACK32_OFF = {}, {}
_PACK16_COLS = 0
_PACK32_COLS = 0
for _k, (_K, _M) in _WSPECS.items():
    if _k in _MMW:
        _PACK16_OFF[_k] = _PACK16_COLS
        _PACK16_COLS += _M
    else:
        _PACK32_OFF[_k] = _PACK32_COLS
        _PACK32_COLS += _M


# ---------------------------------------------------------------- device build
def build_nc(n_loc: int, repeat: int = 1, debug_taps: bool = False):
    """Build + compile the per-core Bass program. n_loc = samples per core."""
    import concourse.bass as bass
    import concourse.bacc as bacc
    import concourse.tile as tile
    import concourse.mybir as mybir
    from contextlib import ExitStack

    f32 = mybir.dt.float32
    f16 = mybir.dt.float16  # compute dtype: full PE rate, 2x DVE modes
    AF = mybir.ActivationFunctionType
    OP = mybir.AluOpType

    nc = bacc.Bacc("TRN2", target_bir_lowering=False, debug=False,
                   enable_asserts=False)
    mm = nc.tensor.matmul

    C = n_loc * O
    n_chunks = C // TC
    assert C % TC == 0

    zT = nc.dram_tensor("zT", (D_IN, C), f16, kind="ExternalInput").ap()
    outT = nc.dram_tensor("outT", (CL, C), f32, kind="ExternalOutput").ap()
    taps = {}
    if debug_taps:
        for tname, (tp, tw) in {"t_stf": (32, TC), "t_aux": (O, TC),
                                "t_a1": (128, TC), "t_m3": (96, TC),
                                "t_att3": (96, TC), "t_alr": (99, TC),
                                "t_am": (96, TC), "t_dyn": (96, TC),
                                "t_hself": (96, TC)}.items():
            taps[tname] = nc.dram_tensor(tname, (tp, tw), f32,
                                         kind="ExternalOutput").ap()

    def tap(name, tile_ap):
        if debug_taps and name in taps:
            nc.gpsimd.dma_start(taps[name][:], tile_ap)
    wp16d = nc.dram_tensor("wpack16", (128, _PACK16_COLS), f16,
                           kind="ExternalInput").ap()
    wp32d = nc.dram_tensor("wpack32", (128, _PACK32_COLS), f32,
                           kind="ExternalInput").ap()

    with tile.TileContext(nc) as tc:
        with ExitStack() as ctx:
            # --- persistent weight tiles (two packed DMAs, per-weight views)
            wpool = ctx.enter_context(tc.tile_pool(name="w", bufs=1))
            wp16 = wpool.tile([128, _PACK16_COLS], f16, tag="wp16", name="wp16")
            nc.sync.dma_start(wp16[:], wp16d[:])
            wp32 = wpool.tile([128, _PACK32_COLS], f32, tag="wp32", name="wp32")
            nc.sync.dma_start(wp32[:], wp32d[:])
            W = {}
            for k, (K_, M_) in _WSPECS.items():
                r0 = _PACK_ROWOFF.get(k, 0)
                if k in _MMW:
                    W[k] = wp16[r0:r0+K_, _PACK16_OFF[k]:_PACK16_OFF[k]+M_]
                else:
                    W[k] = wp32[r0:r0+K_, _PACK32_OFF[k]:_PACK32_OFF[k]+M_]

            # persistent tensors
            pers = ctx.enter_context(tc.tile_pool(name="pers", bufs=1))
            stf = pers.tile([32, C], f16, tag="stf", name="stf")
            aux = pers.tile([O, C], f16, tag="aux", name="aux")  # rows 0:8 = dist_j

            # --- pools
            p_zt = ctx.enter_context(tc.tile_pool(name="zt", bufs=2))
            p_dt8 = ctx.enter_context(tc.tile_pool(name="dt8", bufs=2))
            p_sq = ctx.enter_context(tc.tile_pool(name="sq", bufs=3))
            p_hself = ctx.enter_context(tc.tile_pool(name="hself", bufs=2))
            p_x = ctx.enter_context(tc.tile_pool(name="x", bufs=3))
            p_a1 = ctx.enter_context(tc.tile_pool(name="a1", bufs=6))
            p_m3 = ctx.enter_context(tc.tile_pool(name="m3", bufs=3))
            p_al = ctx.enter_context(tc.tile_pool(name="al", bufs=3))
            p_am = ctx.enter_context(tc.tile_pool(name="am", bufs=3))
            p_ep = ctx.enter_context(tc.tile_pool(name="ep", bufs=4))
            p_out = ctx.enter_context(tc.tile_pool(name="outs", bufs=2))

            ps_A = ctx.enter_context(tc.tile_pool(name="psA", bufs=3, space="PSUM"))
            ps_B = ctx.enter_context(tc.tile_pool(name="psB", bufs=2, space="PSUM"))
            ps_S = ctx.enter_context(tc.tile_pool(name="psS", bufs=2, space="PSUM"))
            ps_D = ctx.enter_context(tc.tile_pool(name="psD", bufs=1, space="PSUM"))

            for _rep in range(repeat):
              # ============ PASS 0: encoder + dist rows (sqrt table resident)
              for cc in range(n_chunks):
                cs = cc * TC
                s0 = cs // O          # first sample of chunk
                ns = TC // O          # samples per chunk
                zt = p_zt.tile([D_IN, TC], f16, tag="zt", name="zt")
                nc.sync.dma_start(zt[:], zT[:, cs:cs+TC])
                ps_enc = ps_S.tile([33, TC], f32, tag="ps_s", name="ps_enc")
                mm(ps_enc[0:32, :], W["wenc"][:], zt[:], start=True, stop=True)
                nc.vector.tensor_scalar(stf[0:32, cs:cs+TC], ps_enc[0:32, :],
                                        W["benc"][:], None, op0=OP.add)
                st_soi = stf[0:32, :].rearrange("p (s i) -> p s i", i=O)
                dt8 = p_dt8.tile([1, O * TC], f16, tag="dt8", name="dt8")
                for j in range(O):
                    diff = p_sq.tile([32, TC], f16, tag="diff", name="diff")
                    diff_eng = nc.gpsimd if j in (2, 6) else nc.vector
                    diff_eng.tensor_tensor(
                        diff[:].rearrange("p (s i) -> p s i", i=O),
                        st_soi[:, s0:s0+ns, :],
                        st_soi[:, s0:s0+ns, j:j+1].broadcast_to((32, ns, O)),
                        op=OP.subtract)
                    dsq = p_sq.tile([32, TC], f16, tag="dsq", name="dsq")
                    if j in (0, 4):
                        nc.scalar.activation(dsq[:], diff[:], AF.Square)
                    else:
                        nc.vector.tensor_tensor(dsq[:], diff[:], diff[:], op=OP.mult)
                    ps_d2 = ps_S.tile([33, TC], f32, tag="ps_s", name="ps_d2")
                    mm(ps_d2[0:1, :], W["ones32"][:], dsq[:], start=True, stop=True)
                    nc.scalar.activation(dt8[0:1, j*TC:(j+1)*TC], ps_d2[0:1, :],
                                         AF.Sqrt, bias=W["epsb"][:])
                # scatter the 8 dist rows into aux via DMA (engine writes at
                # partition j would violate the partition-alignment rule)
                for j in range(O):
                    nc.sync.dma_start(aux[j:j+1, cs:cs+TC],
                                      dt8[0:1, j*TC:(j+1)*TC])
                if cc == 0:
                    tap("t_stf", stf[0:32, 0:TC])
                    tap("t_aux", aux[0:O, 0:TC])

              # ============ MAIN PASS (sigmoid table resident)
              for cc in range(n_chunks):
                cs = cc * TC
                s0 = cs // O
                ns = TC // O
                st_soi = stf[0:32, :].rearrange("p (s i) -> p s i", i=O)
                hself = p_hself.tile([96, TC], f16, tag="hself", name="hself")
                ps_h = ps_B.tile([128, TC], f32, tag="ps_b", name="ps_h")
                mm(ps_h[0:96, :], W["wself0s"][:], stf[0:32, cs:cs+TC], start=True, stop=True)
                nc.vector.tensor_scalar(hself[:], ps_h[0:96, :], W["sb0s"][:], 0.0,
                                        op0=OP.add, op1=OP.max)

                dyn_acc = ps_D.tile([96, TC], f32, tag="ps_d", name="dyn_acc")

                # 3-stage software pipeline over j-slabs: stage A (psA+a1) for
                # slab ja, stage B (psB+evac) for ja-1, stage C (psC/alr/am/
                # aggregate) for ja-2 -- keeps every engine's program order
                # free of head-of-line dependency stalls.
                stA, stB = {}, {}
                for ja in range(O + 3):
                    if ja <= O:
                        diag = ja == O
                        if not diag:
                            xj33 = p_x.tile([33, TC], f16, tag="x", name="xj33")
                            nc.gpsimd.tensor_copy(
                                xj33[0:32, :].rearrange("p (s i) -> p s i", i=O),
                                st_soi[:, s0:s0+ns, ja:ja+1].broadcast_to((32, ns, O)))
                            nc.sync.dma_start(xj33[32:33, :], aux[ja:1+ja, cs:cs+TC])
                        a1s = []
                        for c in range(3):
                            psA = ps_A.tile([128, TC], f32, tag="psA", name="psA")
                            if diag:
                                mm(psA[:], W[f"w1d_{c}"][:], stf[0:32, cs:cs+TC],
                                   start=True, stop=True)
                            else:
                                mm(psA[:], W[f"w1pa_{c}"][:], stf[0:32, cs:cs+TC],
                                   start=True, stop=False)
                                mm(psA[:], W[f"w1pbc_{c}"][:], xj33[:],
                                   start=False, stop=True)
                            a1 = p_a1.tile([128, TC], f16, tag="a1", name="a1")
                            nc.scalar.activation(a1[:], psA[:], AF.Relu,
                                                 bias=W[f"b1_{c}"][:])
                            a1s.append(a1)
                        if cc == 0 and ja == 0:
                            tap("t_a1", a1s[0][:])
                        stA[ja] = (a1s, diag)

                    jb = ja - 1
                    if 0 <= jb <= O:
                        a1s, diag = stA.pop(jb)
                        psBm = ps_B.tile([128, TC], f32, tag="ps_b", name="psBm")
                        psBt = ps_B.tile([128, TC], f32, tag="ps_b", name="psBt")
                        for c in range(3):
                            mm(psBm[32*c:32*c+32, :], W[f"w2pm_{c}"][:], a1s[c][:],
                               start=True, stop=True)
                            mm(psBt[32*c:32*c+32, :], W[f"w2pt_{c}"][:], a1s[c][:],
                               start=True, stop=True)
                        m3 = p_m3.tile([96, TC], f16, tag="m3", name="m3")
                        nc.vector.tensor_scalar(m3[:], psBm[0:96, :], W["b2m"][:], 0.0,
                                                op0=OP.add, op1=OP.max)
                        att3 = p_m3.tile([96, TC], f16, tag="att3", name="att3")
                        nc.vector.tensor_scalar(att3[:], psBt[0:96, :], W["b2t"][:], 0.0,
                                                op0=OP.add, op1=OP.max)
                        if cc == 0 and jb == 0:
                            tap("t_m3", m3[:])
                            tap("t_att3", att3[:])
                        stB[jb] = (m3, att3, diag)

                    jc = ja - 2
                    if 0 <= jc <= O:
                        m3, att3, diag = stB.pop(jc)
                        psC = ps_S.tile([99, TC], f32, tag="ps_s", name="psC")
                        mm(psC[:], W["w3at"][:], att3[:], start=True, stop=True)
                        alr = p_al.tile([99, TC], f16, tag="al", name="alr")
                        nc.scalar.activation(alr[:], psC[:], AF.Sigmoid,
                                             bias=W["ab2r9"][:])
                        am = p_am.tile([96, TC], f16, tag="am", name="am")
                        nc.vector.tensor_tensor(am[:], m3[:], alr[0:96, :], op=OP.mult)
                        if cc == 0 and jc == 0:
                            tap("t_alr", alr[:])
                            tap("t_am", am[:])
                        mm(dyn_acc[:], W["w3rbdn" if diag else "w3rbd"][:], am[:],
                           start=(jc == 0), stop=False, skip_group_check=True)
                        mm(dyn_acc[:], W["w3rb3qn" if diag else "w3rb3q"][:], alr[64:99, :],
                           start=False, stop=False, skip_group_check=True)

                # self-dynamics into the same accumulator, then evacuate
                mm(dyn_acc[:], W["wself1bd"][:], hself[:],
                   start=False, stop=True, skip_group_check=True)
                dyn = p_ep.tile([96, TC], f16, tag="ep", name="dyn")
                nc.vector.tensor_scalar(dyn[:], dyn_acc[:], W["dynb"][:], None, op0=OP.add)
                if cc == 0:
                    tap("t_hself", hself[:])
                    tap("t_dyn", dyn[:])

                # ---- affector + out + agg chains
                cur = dyn
                for k in range(3):
                    psE = ps_D.tile([96, TC], f32, tag="ps_d", name="psE")
                    mm(psE[:], W[f"waff{k}bd"][:], cur[:], start=True, stop=True)
                    nxt = p_ep.tile([96, TC], f16, tag="ep", name="nxt")
                    if k == 0:
                        nc.scalar.activation(nxt[:], psE[:], AF.Relu, bias=W["fb0s"][:])
                    elif k == 1:
                        nc.vector.tensor_scalar(nxt[:], psE[:], W[f"fb{k}s"][:], 0.0,
                                                op0=OP.add, op1=OP.max)
                    else:
                        nc.vector.tensor_scalar(nxt[:], psE[:], W[f"fb{k}s"][:], None, op0=OP.add)
                    cur = nxt
                psO = ps_D.tile([96, TC], f32, tag="ps_d", name="psO")
                mm(psO[:], W["wow0abd"][:], cur[:], start=True, stop=False)
                mm(psO[:], W["wow0ss"][:], stf[0:32, cs:cs+TC], start=False, stop=True)
                o0 = p_ep.tile([96, TC], f16, tag="ep", name="o0")
                nc.scalar.activation(o0[:], psO[:], AF.Relu, bias=W["ob0s"][:])
                # ow1 has no following nonlinearity: fused into wagg1c on host
                psG = ps_S.tile([33, TC], f32, tag="ps_s", name="psG")
                mm(psG[0:32, :], W["wagg1c"][:], o0[:], start=True, stop=True)
                h = p_ep.tile([32, TC], f16, tag="ep", name="h")
                nc.scalar.activation(h[:], psG[0:32, :], AF.Relu, bias=W["bagg1p"][:])
                psG2 = ps_S.tile([33, TC], f32, tag="ps_s", name="psG2")
                mm(psG2[0:32, :], W["wagg2"][:], h[:], start=True, stop=True)
                ot = p_out.tile([32, TC], f32, tag="ot", name="ot")
                nc.vector.tensor_scalar(ot[:], psG2[0:32, :], W["bagg2"][:], None, op0=OP.add)
                nc.sync.dma_start(outT[:, cs:cs+TC], ot[:])

    nc.compile()
    return nc


# ---------------------------------------------------------------- host runner
_CACHE = {}


def _make_runner(nc, n_cores=N_CORES):
    import jax
    import numpy as _np
    import concourse.mybir as mybir
    from concourse import bass2jax
    from jax.sharding import Mesh, PartitionSpec
    from jax.experimental.shard_map import shard_map

    bass2jax.install_neuronx_cc_hook()
    partition_name = nc.partition_id_tensor.name if nc.partition_id_tensor else None
    in_names, out_names, out_avals, zero_shapes = [], [], [], []
    for alloc in nc.m.functions[0].allocations:
        if not isinstance(alloc, mybir.MemoryLocationSet):
            continue
        name = alloc.memorylocations[0].name
        if alloc.kind == "ExternalInput":
            if name != partition_name:
                in_names.append(name)
        elif alloc.kind == "ExternalOutput":
            out_names.append(name)
            shape = tuple(alloc.tensor_shape)
            dtype = mybir.dt.np(alloc.dtype)
            out_avals.append(jax.core.ShapedArray(shape, dtype))
            zero_shapes.append((shape, dtype))
    n_params = len(in_names)
    n_outs = len(out_avals)
    all_in_names = in_names + out_names + ([partition_name] if partition_name else [])
    donate = tuple(range(n_params, n_params + n_outs))

    def _body(*args):
        operands = list(args)
        if partition_name is not None:
            operands.append(bass2jax.partition_id_tensor())
        outs = bass2jax._bass_exec_p.bind(
            *operands, out_avals=tuple(out_avals), in_names=tuple(all_in_names),
            out_names=tuple(out_names), lowering_input_output_aliases=(),
            sim_require_finite=False, sim_require_nnan=False, nc=nc)
        return tuple(outs)

    devices = jax.devices()[:n_cores]
    mesh = Mesh(_np.asarray(devices), ("core",))
    sharded = jax.jit(
        shard_map(_body, mesh=mesh,
                  in_specs=(PartitionSpec("core"),) * (n_params + n_outs),
                  out_specs=(PartitionSpec("core"),) * n_outs,
                  check_rep=False),
        donate_argnums=donate, keep_unused=True)

    def run(in_maps):
        n_c = len(in_maps)
        per_core = [[_np.asarray(m[name]) for name in in_names] for m in in_maps]
        concat_in = [_np.concatenate([per_core[c][i] for c in range(n_c)], axis=0)
                     for i in range(n_params)]
        concat_zeros = [_np.zeros((n_c * s[0], *s[1:]), d) for s, d in zero_shapes]
        out_arrs = sharded(*concat_in, *concat_zeros)
        jax.block_until_ready(out_arrs)
        return [
            {name: _np.asarray(out_arrs[i]).reshape(n_c, *out_avals[i].shape)[c]
             for i, name in enumerate(out_names)}
            for c in range(n_c)
        ]
    return run


def make_in_maps(inputs: dict, n_loc: int, n_cores: int = N_CORES):
    w = pack_weights(inputs)
    p16 = np.zeros((128, _PACK16_COLS), np.float16)
    p32 = np.zeros((128, _PACK32_COLS), np.float32)
    for k, (K_, M_) in _WSPECS.items():
        r0 = _PACK_ROWOFF.get(k, 0)
        if k in _MMW:
            p16[r0:r0+K_, _PACK16_OFF[k]:_PACK16_OFF[k]+M_] = w[k]
        else:
            p32[r0:r0+K_, _PACK32_OFF[k]:_PACK32_OFF[k]+M_] = w[k]
    z = np.asarray(inputs["z"], np.float32)
    in_maps = []
    for c in range(n_cores):
        zc = z[c*n_loc:(c+1)*n_loc].reshape(n_loc * O, D_IN)
        m = {"wpack16": p16, "wpack32": p32,
             "zT": np.ascontiguousarray(zc.T).astype(np.float16)}
        in_maps.append(m)
    return in_maps


def kernel(**inputs) -> np.ndarray:
    n = inputs["z"].shape[0]
    n_loc = n // N_CORES
    key = ("k", n_loc)
    if key not in _CACHE:
        nc = build_nc(n_loc)
        _CACHE[key] = (nc, _make_runner(nc))
    nc, runner = _CACHE[key]
    res = runner(make_in_maps(inputs, n_loc))
    out = np.concatenate(
        [res[c]["outT"].T.reshape(n_loc, O, CL) for c in range(N_CORES)], axis=0)
    return out


# revision 56
# speedup vs baseline: 8.2970x; 1.9796x over previous
"""Trainium2 Bass kernel for nn_Net_21818433863930 (interaction-network GNN).

Contract: kernel(**inputs) takes FULL unsharded fp32 inputs (z: (8192, 8, 16) plus
all MLP weights), shards batch across 8 NeuronCores (pure data parallel), runs a
Bass/Tile kernel per core, and returns the FULL (8192, 8, 32) fp32 output.

v2 layout (per core, n_loc=1024 samples, C=8192 object-cols, feature-major):
  - SBUF tiles are (features, columns); pair grid processed as 16 chunks x 512
    object-cols x 9 j-slabs (slab 8 = diagonal, aggregated with NEGATED weights
    to realize the (1-eye) mask by cancellation)
  - the three interaction cores' rel/att stages are STACKED: psBm (3x33 rows)
    and psBt (3x32 rows) PSUM tiles collect all cores' hidden layers so the
    relu/sigmoid/mult/aggregate steps run once per j-slab instead of once per
    (j, core) -- block-diagonal lhsT weights do the bookkeeping
  - dist is a 33rd input row of the partner tile (w1pbc = [w1pb; w1pc]); the
    diagonal slab merges both psA matmuls into one (w1d = w1pa + w1pb)
  - pass 0 (sqrt act table) computes encoder + pair distances, writing dist rows
    straight from the Act engine into the persistent aux tile (no DMA roundtrip)
"""
import numpy as np

N, O, CL = 8192, 8, 32
D_IN = 16
EPS = 1e-12
N_CORES = 8
TC = 512  # columns per chunk


# ---------------------------------------------------------------- host packing
def pack_weights(inp: dict) -> dict:
    """Pack reference weights into lhsT/bias arrays the kernel consumes.
    All matmul weights are (K, M) = (in_features, out_features) fp32."""
    f32 = np.float32
    w = {}
    w["wenc"] = np.ascontiguousarray(inp["Wenc"], f32)            # (16, 32)
    w["benc"] = np.ascontiguousarray(inp["benc"].reshape(32, 1), f32)

    for c in range(3):
        rw0, aw0 = inp["rw0"][c], inp["aw0"][c]                   # (65, 64)
        # psA lhsT for the merged (65,TC) pair tile [s_i@0; s_j@32; dist@64]
        w1pa = np.concatenate([rw0[0:32], aw0[0:32]], axis=1)                        # (32,128)
        w1pb = np.concatenate([rw0[32:64], aw0[32:64]], axis=1)                      # (32,128)
        w1pc = np.concatenate([rw0[64:65], aw0[64:65]], axis=1)                      # (1,128)
        w[f"w1f_{c}"] = np.concatenate([w1pa, w1pb, w1pc], axis=0).astype(f32)       # (65,128)
        w[f"w1d_{c}"] = (w1pa + w1pb).astype(f32)                                    # (32,128)
        w[f"b1_{c}"] = np.concatenate([inp["rb0"][c], inp["ab0"][c]]).reshape(128, 1).astype(f32)

        # psBm: a1(128) -> rel-hidden2 (32) written at offset 32c
        w2pm = np.zeros((128, 32), f32)
        w2pm[0:64, :] = inp["rw1"][c]
        w[f"w2pm_{c}"] = w2pm
        # psBt: a1(128) -> att-hidden2 (32) at offset 32c
        w2pt = np.zeros((128, 32), f32)
        w2pt[64:128, :] = inp["aw1"][c]
        w[f"w2pt_{c}"] = w2pt

    # biases for the stacked m3 (96,1) / att3 (96,1) evacuations
    b2m = np.zeros((96, 1), f32)
    b2t = np.zeros((96, 1), f32)
    ab2r9 = np.zeros((99, 1), f32)
    for c in range(3):
        b2m[32*c:32*c+32, 0] = inp["rb1"][c]
        b2t[32*c:32*c+32, 0] = inp["ab1"][c]
        ab2r9[96+c, 0] = float(inp["ab2"][c][0])
        ab2r9[32*c:32*c+32, 0] = float(inp["ab2"][c][0])
    w["b2m"] = b2m
    w["b2t"] = b2t
    w["ab2r9"] = ab2r9

    # psC lhsT: att3 (96) -> 99 rows: rows 32c = 32 broadcast copies of core
    # c's logit (for the am product), rows 96:99 = per-core scalar logits
    w3at = np.zeros((96, 99), f32)
    for c in range(3):
        w3at[32*c:32*c+32, 96+c] = inp["aw2"][c].reshape(32)
        w3at[32*c:32*c+32, 32*c:32*c+32] = np.tile(inp["aw2"][c].reshape(32, 1), (1, 32))
    w["w3at"] = w3at

    # aggregation lhsT: am (99 = [m3*sig (96); sig (3)]) -> dyn (96):
    # block-diag rw2 over the product rows, rb2 over the sigma rows
    w3rbd = np.zeros((99, 96), f32)
    for c in range(3):
        w3rbd[32*c:32*c+32, 32*c:32*c+32] = inp["rw2"][c]
        w3rbd[96+c, 32*c:32*c+32] = inp["rb2"][c]
    w["w3rbd"] = w3rbd
    w["w3rbdn"] = -w3rbd

    # pass0: block-diag column-of-ones reducer for 4 stacked j-slabs
    ones_bd4 = np.zeros((128, 4), f32)
    for q in range(4):
        ones_bd4[32*q:32*q+32, q] = 1.0
    w["ones_bd4"] = ones_bd4

    def bd(ms):  # block-diag of three (32, 32)
        out = np.zeros((96, 96), f32)
        for c in range(3):
            out[32*c:32*c+32, 32*c:32*c+32] = ms[c]
        return out

    w["wself0s"] = np.concatenate([inp["sw0"][c] for c in range(3)], axis=1).astype(f32)  # (32, 96)
    w["sb0s"] = np.concatenate([inp["sb0"][c] for c in range(3)]).reshape(96, 1).astype(f32)
    w["wself1bd"] = bd([inp["sw1"][c] for c in range(3)])
    w["dynb"] = np.concatenate([inp["sb1"][c] for c in range(3)]).reshape(96, 1).astype(f32)

    for k, (wn, bn) in enumerate([("fw0", "fb0"), ("fw1", "fb1"), ("fw2", "fb2")]):
        w[f"waff{k}bd"] = bd([inp[wn][c] for c in range(3)])
        w[f"fb{k}s"] = np.concatenate([inp[bn][c] for c in range(3)]).reshape(96, 1).astype(f32)

    # out-MLP layer0 lhsT for the (128,TC) tile [aff2(0:96); stf(96:128)]
    w["wow0c"] = np.concatenate(
        [bd([inp["ow0"][c][0:32] for c in range(3)]),
         np.concatenate([inp["ow0"][c][32:64] for c in range(3)], axis=1)],
        axis=0).astype(f32)                                        # (128, 96)
    w["ob0s"] = np.concatenate([inp["ob0"][c] for c in range(3)]).reshape(96, 1).astype(f32)

    ob1cat = np.concatenate([inp["ob1"][c] for c in range(3)])     # (96,)
    # ow1 (per-core, no nonlinearity after) fused into agg1: (ow1_bd @ Wagg1)
    w["wagg1c"] = (bd([inp["ow1"][c] for c in range(3)]).astype(np.float64)
                   @ np.asarray(inp["Wagg1"], np.float64)).astype(f32)  # (96, 32)
    w["bagg1p"] = (inp["Wagg1"].T @ ob1cat + inp["bagg1"]).reshape(32, 1).astype(f32)
    w["wagg2"] = np.ascontiguousarray(inp["Wagg2"], f32)           # (32, 32)
    w["bagg2"] = np.ascontiguousarray(inp["bagg2"].reshape(32, 1), f32)

    w["epsb"] = np.full((4, 1), EPS, f32)
    return w


# weights consumed by the PE (matmul lhsT / rhs) -> shipped as fp16
_MMW = (["wenc", "wself0s", "wself1bd", "waff0bd", "waff1bd", "waff2bd",
         "wow0c", "wagg1c", "wagg2", "ones_bd4",
         "w3at", "w3rbd", "w3rbdn"]
        + [f"w1f_{c}" for c in range(3)]
        + [f"w1d_{c}" for c in range(3)] + [f"w2pm_{c}" for c in range(3)]
        + [f"w2pt_{c}" for c in range(3)])

_WSPECS = {
    "wenc": (16, 32), "benc": (32, 1),
    "wself0s": (32, 96), "sb0s": (96, 1), "wself1bd": (96, 96), "dynb": (96, 1),
    "waff0bd": (96, 96), "fb0s": (96, 1), "waff1bd": (96, 96), "fb1s": (96, 1),
    "waff2bd": (96, 96), "fb2s": (96, 1),
    "wow0c": (128, 96), "ob0s": (96, 1),
    "wagg1c": (96, 32), "bagg1p": (32, 1), "wagg2": (32, 32), "bagg2": (32, 1),
    "ones_bd4": (128, 4), "epsb": (4, 1),
    "b2m": (96, 1), "b2t": (96, 1), "ab2r9": (99, 1),
    "w3at": (96, 99), "w3rbd": (99, 96), "w3rbdn": (99, 96),
}
# weights consumed as lhsT at a nonzero partition base (must match rhs base)
_PACK_ROWOFF = {}
for _c in range(3):
    _WSPECS[f"w1f_{_c}"] = (65, 128)
    _WSPECS[f"w1d_{_c}"] = (32, 128)
    _WSPECS[f"b1_{_c}"] = (128, 1)
    _WSPECS[f"w2pm_{_c}"] = (128, 32)
    _WSPECS[f"w2pt_{_c}"] = (128, 32)

# packed-weight column offsets: one fp16 pack and one fp32 pack, each a single
# (128, ncols) dram tensor -> single DMA (the per-tensor path costs ~565ns of
# DMA queue time per transfer, ~35us of startup for ~60 weights)
_PACK16_OFF, _PACK32_OFF = {}, {}
_PACK16_COLS = 0
_PACK32_COLS = 0
for _k, (_K, _M) in _WSPECS.items():
    if _k in _MMW:
        _PACK16_OFF[_k] = _PACK16_COLS
        _PACK16_COLS += _M
    else:
        _PACK32_OFF[_k] = _PACK32_COLS
        _PACK32_COLS += _M


# ---------------------------------------------------------------- device build
def build_nc(n_loc: int, repeat: int = 1, debug_taps: bool = False,
             parts: tuple = ("pass0", "main")):
    """Build + compile the per-core Bass program. n_loc = samples per core."""
    import concourse.bass as bass
    import concourse.bacc as bacc
    import concourse.tile as tile
    import concourse.mybir as mybir
    from contextlib import ExitStack

    f32 = mybir.dt.float32
    f16 = mybir.dt.float16  # compute dtype: full PE rate, 2x DVE modes
    AF = mybir.ActivationFunctionType
    OP = mybir.AluOpType

    nc = bacc.Bacc("TRN2", target_bir_lowering=False, debug=False,
                   enable_asserts=False)
    mm = nc.tensor.matmul

    C = n_loc * O
    n_chunks = C // TC
    assert C % TC == 0

    zT = nc.dram_tensor("zT", (D_IN, C), f16, kind="ExternalInput").ap()
    outT = nc.dram_tensor("outT", (CL, C), f32, kind="ExternalOutput").ap()
    taps = {}
    if debug_taps:
        for tname, (tp, tw) in {"t_stf": (32, TC), "t_aux": (O, TC),
                                "t_a1": (128, TC), "t_m3": (96, TC),
                                "t_att3": (96, TC), "t_alr": (99, TC),
                                "t_am": (96, TC), "t_dyn": (96, TC),
                                "t_hself": (96, TC)}.items():
            taps[tname] = nc.dram_tensor(tname, (tp, tw), f32,
                                         kind="ExternalOutput").ap()

    def tap(name, tile_ap):
        if debug_taps and name in taps:
            nc.gpsimd.dma_start(taps[name][:], tile_ap)
    wp16d = nc.dram_tensor("wpack16", (128, _PACK16_COLS), f16,
                           kind="ExternalInput").ap()
    wp32d = nc.dram_tensor("wpack32", (128, _PACK32_COLS), f32,
                           kind="ExternalInput").ap()

    with tile.TileContext(nc) as tc:
        with ExitStack() as ctx:
            # --- persistent weight tiles (two packed DMAs, per-weight views)
            wpool = ctx.enter_context(tc.tile_pool(name="w", bufs=1))
            wp16 = wpool.tile([128, _PACK16_COLS], f16, tag="wp16", name="wp16")
            nc.sync.dma_start(wp16[:], wp16d[:])
            wp32 = wpool.tile([128, _PACK32_COLS], f32, tag="wp32", name="wp32")
            nc.sync.dma_start(wp32[:], wp32d[:])
            W = {}
            for k, (K_, M_) in _WSPECS.items():
                r0 = _PACK_ROWOFF.get(k, 0)
                if k in _MMW:
                    W[k] = wp16[r0:r0+K_, _PACK16_OFF[k]:_PACK16_OFF[k]+M_]
                else:
                    W[k] = wp32[r0:r0+K_, _PACK32_OFF[k]:_PACK32_OFF[k]+M_]

            # persistent tensors
            pers = ctx.enter_context(tc.tile_pool(name="pers", bufs=1))
            stf = pers.tile([32, C], f16, tag="stf", name="stf")
            aux = pers.tile([O, C], f16, tag="aux", name="aux")  # rows 0:8 = dist_j

            # --- pools
            p_zt = ctx.enter_context(tc.tile_pool(name="zt", bufs=2))
            p_dt8 = ctx.enter_context(tc.tile_pool(name="dt8", bufs=2))
            p_sq = ctx.enter_context(tc.tile_pool(name="sq", bufs=3))
            p_hself = ctx.enter_context(tc.tile_pool(name="hself", bufs=2))
            p_x = ctx.enter_context(tc.tile_pool(name="x", bufs=1))
            p_a1 = ctx.enter_context(tc.tile_pool(name="a1", bufs=6))
            p_m3 = ctx.enter_context(tc.tile_pool(name="m3", bufs=3))
            p_al = ctx.enter_context(tc.tile_pool(name="al", bufs=3))
            p_am = ctx.enter_context(tc.tile_pool(name="am", bufs=3))
            p_ep = ctx.enter_context(tc.tile_pool(name="ep", bufs=4))
            p_out = ctx.enter_context(tc.tile_pool(name="outs", bufs=2))

            ps_A = ctx.enter_context(tc.tile_pool(name="psA", bufs=3, space="PSUM"))
            ps_B = ctx.enter_context(tc.tile_pool(name="psB", bufs=2, space="PSUM"))
            ps_S = ctx.enter_context(tc.tile_pool(name="psS", bufs=2, space="PSUM"))
            ps_D = ctx.enter_context(tc.tile_pool(name="psD", bufs=1, space="PSUM"))

            if "pass0" not in parts:
                # timing-only variant: give stf/aux writers
                nc.vector.memset(stf[:], 0.5)
                nc.vector.memset(aux[:], 1.0)

            for _rep in range(repeat):
              # ============ PASS 0: encoder + dist rows (sqrt table resident)
              for cc in range(n_chunks if "pass0" in parts else 0):
                cs = cc * TC
                s0 = cs // O          # first sample of chunk
                ns = TC // O          # samples per chunk
                zt = p_zt.tile([D_IN, TC], f16, tag="zt", name="zt")
                nc.sync.dma_start(zt[:], zT[:, cs:cs+TC])
                ps_enc = ps_S.tile([33, TC], f32, tag="ps_s", name="ps_enc")
                mm(ps_enc[0:32, :], W["wenc"][:], zt[:], start=True, stop=True)
                nc.vector.tensor_scalar(stf[0:32, cs:cs+TC], ps_enc[0:32, :],
                                        W["benc"][:], None, op0=OP.add)
                st_soi = stf[0:32, :].rearrange("p (s i) -> p s i", i=O)
                dt8 = p_dt8.tile([4, 2 * TC], f16, tag="dt8", name="dt8")
                for g in range(2):
                    # 4 j-slabs stacked at quadrant offsets -> one dsq / one
                    # reduction matmul / one sqrt for all 4
                    diff4 = p_sq.tile([128, TC], f16, tag="diff", name="diff4")
                    for jj in range(4):
                        j = 4 * g + jj
                        diff_eng = nc.gpsimd if jj == 1 else nc.vector
                        diff_eng.tensor_tensor(
                            diff4[32*jj:32*jj+32, :].rearrange("p (s i) -> p s i", i=O),
                            st_soi[:, s0:s0+ns, :],
                            st_soi[:, s0:s0+ns, j:j+1].broadcast_to((32, ns, O)),
                            op=OP.subtract)
                    dsq4 = p_sq.tile([128, TC], f16, tag="dsq", name="dsq4")
                    nc.vector.tensor_tensor(dsq4[:], diff4[:], diff4[:], op=OP.mult)
                    ps_d2 = ps_S.tile([33, TC], f32, tag="ps_s", name="ps_d2")
                    mm(ps_d2[0:4, :], W["ones_bd4"][:], dsq4[:], start=True, stop=True)
                    nc.scalar.activation(dt8[0:4, g*TC:(g+1)*TC], ps_d2[0:4, :],
                                         AF.Sqrt, bias=W["epsb"][:])
                # scatter the 8 dist rows into aux via DMA (engine writes at
                # partition j would violate the partition-alignment rule)
                for j in range(O):
                    nc.sync.dma_start(aux[j:j+1, cs:cs+TC],
                                      dt8[j % 4:j % 4 + 1, (j // 4)*TC:(j // 4 + 1)*TC])
                if cc == 0:
                    tap("t_stf", stf[0:32, 0:TC])
                    tap("t_aux", aux[0:O, 0:TC])

              if "main" not in parts:
                  # timing-only variant: drain stf/aux so the tile framework
                  # sees a consumer
                  nc.gpsimd.dma_start(outT[0:32, :], stf[0:32, :])
                  nc.gpsimd.dma_start(outT[0:O, :], aux[0:O, :])

              # ============ MAIN PASS (sigmoid table resident)
              for cc in range(n_chunks if "main" in parts else 0):
                cs = cc * TC
                s0 = cs // O
                ns = TC // O
                st_soi = stf[0:32, :].rearrange("p (s i) -> p s i", i=O)
                hself = p_hself.tile([96, TC], f16, tag="hself", name="hself")
                ps_h = ps_B.tile([128, TC], f32, tag="ps_b", name="ps_h")
                mm(ps_h[0:96, :], W["wself0s"][:], stf[0:32, cs:cs+TC], start=True, stop=True)
                nc.vector.tensor_scalar(hself[:], ps_h[0:96, :], W["sb0s"][:], 0.0,
                                        op0=OP.add, op1=OP.max)

                dyn_acc = ps_D.tile([96, TC], f32, tag="ps_d", name="dyn_acc")

                # merged pair tiles [s_i@0; s_j@32; dist@64]: 3 rotating slots,
                # s_i rows filled once per chunk by DMA (engines stay free)
                xfs = [p_x.tile([65, TC], f16, tag=f"x{i}", name=f"xf{i}")
                       for i in range(3)]
                for i in range(3):
                    nc.sync.dma_start(xfs[i][0:32, :], stf[0:32, cs:cs+TC])

                # 3-stage software pipeline over j-slabs: stage A (psA+a1) for
                # slab ja, stage B (psB+evac) for ja-1, stage C (psC/alr/am/
                # aggregate) for ja-2 -- keeps every engine's program order
                # free of head-of-line dependency stalls.
                stA, stB = {}, {}
                for ja in range(O + 3):
                    if ja <= O:
                        diag = ja == O
                        if not diag:
                            xf = xfs[ja % 3]
                            nc.vector.tensor_copy(
                                xf[32:64, :].rearrange("p (s i) -> p s i", i=O),
                                st_soi[:, s0:s0+ns, ja:ja+1].broadcast_to((32, ns, O)))
                            nc.sync.dma_start(xf[64:65, :], aux[ja:1+ja, cs:cs+TC])
                        a1s = []
                        for c in range(3):
                            psA = ps_A.tile([128, TC], f32, tag="psA", name="psA")
                            if diag:
                                mm(psA[:], W[f"w1d_{c}"][:], stf[0:32, cs:cs+TC],
                                   start=True, stop=True)
                            else:
                                mm(psA[:], W[f"w1f_{c}"][:], xf[:], start=True, stop=True)
                            a1 = p_a1.tile([128, TC], f16, tag="a1", name="a1")
                            nc.scalar.activation(a1[:], psA[:], AF.Relu,
                                                 bias=W[f"b1_{c}"][:])
                            a1s.append(a1)
                        if cc == 0 and ja == 0:
                            tap("t_a1", a1s[0][:])
                        stA[ja] = (a1s, diag)

                    jb = ja - 1
                    if 0 <= jb <= O:
                        a1s, diag = stA.pop(jb)
                        psBm = ps_B.tile([128, TC], f32, tag="ps_b", name="psBm")
                        psBt = ps_B.tile([128, TC], f32, tag="ps_b", name="psBt")
                        for c in range(3):
                            mm(psBm[32*c:32*c+32, :], W[f"w2pm_{c}"][:], a1s[c][:],
                               start=True, stop=True)
                            mm(psBt[32*c:32*c+32, :], W[f"w2pt_{c}"][:], a1s[c][:],
                               start=True, stop=True)
                        m3 = p_m3.tile([96, TC], f16, tag="m3", name="m3")
                        nc.vector.tensor_scalar(m3[:], psBm[0:96, :], W["b2m"][:], 0.0,
                                                op0=OP.add, op1=OP.max)
                        att3 = p_m3.tile([96, TC], f16, tag="att3", name="att3")
                        nc.vector.tensor_scalar(att3[:], psBt[0:96, :], W["b2t"][:], 0.0,
                                                op0=OP.add, op1=OP.max)
                        if cc == 0 and jb == 0:
                            tap("t_m3", m3[:])
                            tap("t_att3", att3[:])
                        stB[jb] = (m3, att3, diag)

                    jc = ja - 2
                    if 0 <= jc <= O:
                        m3, att3, diag = stB.pop(jc)
                        psC = ps_S.tile([99, TC], f32, tag="ps_s", name="psC")
                        mm(psC[:], W["w3at"][:], att3[:], start=True, stop=True)
                        alr = p_al.tile([99, TC], f16, tag="al", name="alr")
                        nc.scalar.activation(alr[:], psC[:], AF.Sigmoid,
                                             bias=W["ab2r9"][:])
                        am = p_am.tile([99, TC], f16, tag="am", name="am")
                        nc.vector.tensor_tensor(am[0:96, :], m3[:], alr[0:96, :], op=OP.mult)
                        nc.vector.tensor_copy(am[96:99, :], alr[96:99, :])
                        if cc == 0 and jc == 0:
                            tap("t_alr", alr[:])
                            tap("t_am", am[0:96, :])
                        mm(dyn_acc[:], W["w3rbdn" if diag else "w3rbd"][:], am[:],
                           start=(jc == 0), stop=False, skip_group_check=True)

                # self-dynamics into the same accumulator, then evacuate
                mm(dyn_acc[:], W["wself1bd"][:], hself[:],
                   start=False, stop=True, skip_group_check=True)
                dyn = p_ep.tile([96, TC], f16, tag="ep", name="dyn")
                nc.vector.tensor_scalar(dyn[:], dyn_acc[:], W["dynb"][:], None, op0=OP.add)
                if cc == 0:
                    tap("t_hself", hself[:])
                    tap("t_dyn", dyn[:])

                # ---- affector + out + agg chains
                cur = dyn
                for k in range(3):
                    psE = ps_D.tile([96, TC], f32, tag="ps_d", name="psE")
                    mm(psE[:], W[f"waff{k}bd"][:], cur[:], start=True, stop=True)
                    if k < 2:
                        nxt = p_ep.tile([96, TC], f16, tag="ep", name="nxt")
                        if k == 0:
                            nc.scalar.activation(nxt[:], psE[:], AF.Relu, bias=W["fb0s"][:])
                        else:
                            nc.vector.tensor_scalar(nxt[:], psE[:], W[f"fb{k}s"][:], 0.0,
                                                    op0=OP.add, op1=OP.max)
                    else:
                        # aff2 evac into rows 0:96 of a (128,TC) tile whose rows
                        # 96:128 carry a copy of stf, so the out-MLP layer-0 is
                        # one K=128 matmul (wow0c)
                        nxt = p_ep.tile([128, TC], f16, tag="ep2", name="nxt2")
                        nc.vector.tensor_scalar(nxt[0:96, :], psE[:], W[f"fb{k}s"][:],
                                                None, op0=OP.add)
                        nc.sync.dma_start(nxt[96:128, :], stf[0:32, cs:cs+TC])
                    cur = nxt
                psO = ps_D.tile([96, TC], f32, tag="ps_d", name="psO")
                mm(psO[:], W["wow0c"][:], cur[:], start=True, stop=True)
                o0 = p_ep.tile([96, TC], f16, tag="ep", name="o0")
                nc.scalar.activation(o0[:], psO[:], AF.Relu, bias=W["ob0s"][:])
                # ow1 has no following nonlinearity: fused into wagg1c on host
                psG = ps_S.tile([33, TC], f32, tag="ps_s", name="psG")
                mm(psG[0:32, :], W["wagg1c"][:], o0[:], start=True, stop=True)
                h = p_ep.tile([32, TC], f16, tag="ep", name="h")
                nc.scalar.activation(h[:], psG[0:32, :], AF.Relu, bias=W["bagg1p"][:])
                psG2 = ps_S.tile([33, TC], f32, tag="ps_s", name="psG2")
                mm(psG2[0:32, :], W["wagg2"][:], h[:], start=True, stop=True)
                ot = p_out.tile([32, TC], f32, tag="ot", name="ot")
                nc.vector.tensor_scalar(ot[:], psG2[0:32, :], W["bagg2"][:], None, op0=OP.add)
                nc.sync.dma_start(outT[:, cs:cs+TC], ot[:])

    nc.compile()
    return nc


# ---------------------------------------------------------------- host runner
_CACHE = {}


def _make_runner(nc, n_cores=N_CORES):
    import jax
    import numpy as _np
    import concourse.mybir as mybir
    from concourse import bass2jax
    from jax.sharding import Mesh, PartitionSpec
    from jax.experimental.shard_map import shard_map

    bass2jax.install_neuronx_cc_hook()
    partition_name = nc.partition_id_tensor.name if nc.partition_id_tensor else None
    in_names, out_names, out_avals, zero_shapes = [], [], [], []
    for alloc in nc.m.functions[0].allocations:
        if not isinstance(alloc, mybir.MemoryLocationSet):
            continue
        name = alloc.memorylocations[0].name
        if alloc.kind == "ExternalInput":
            if name != partition_name:
                in_names.append(name)
        elif alloc.kind == "ExternalOutput":
            out_names.append(name)
            shape = tuple(alloc.tensor_shape)
            dtype = mybir.dt.np(alloc.dtype)
            out_avals.append(jax.core.ShapedArray(shape, dtype))
            zero_shapes.append((shape, dtype))
    n_params = len(in_names)
    n_outs = len(out_avals)
    all_in_names = in_names + out_names + ([partition_name] if partition_name else [])
    donate = tuple(range(n_params, n_params + n_outs))

    def _body(*args):
        operands = list(args)
        if partition_name is not None:
            operands.append(bass2jax.partition_id_tensor())
        outs = bass2jax._bass_exec_p.bind(
            *operands, out_avals=tuple(out_avals), in_names=tuple(all_in_names),
            out_names=tuple(out_names), lowering_input_output_aliases=(),
            sim_require_finite=False, sim_require_nnan=False, nc=nc)
        return tuple(outs)

    devices = jax.devices()[:n_cores]
    mesh = Mesh(_np.asarray(devices), ("core",))
    sharded = jax.jit(
        shard_map(_body, mesh=mesh,
                  in_specs=(PartitionSpec("core"),) * (n_params + n_outs),
                  out_specs=(PartitionSpec("core"),) * n_outs,
                  check_rep=False),
        donate_argnums=donate, keep_unused=True)

    def run(in_maps):
        n_c = len(in_maps)
        per_core = [[_np.asarray(m[name]) for name in in_names] for m in in_maps]
        concat_in = [_np.concatenate([per_core[c][i] for c in range(n_c)], axis=0)
                     for i in range(n_params)]
        concat_zeros = [_np.zeros((n_c * s[0], *s[1:]), d) for s, d in zero_shapes]
        out_arrs = sharded(*concat_in, *concat_zeros)
        jax.block_until_ready(out_arrs)
        return [
            {name: _np.asarray(out_arrs[i]).reshape(n_c, *out_avals[i].shape)[c]
             for i, name in enumerate(out_names)}
            for c in range(n_c)
        ]
    return run


def make_in_maps(inputs: dict, n_loc: int, n_cores: int = N_CORES):
    w = pack_weights(inputs)
    p16 = np.zeros((128, _PACK16_COLS), np.float16)
    p32 = np.zeros((128, _PACK32_COLS), np.float32)
    for k, (K_, M_) in _WSPECS.items():
        r0 = _PACK_ROWOFF.get(k, 0)
        if k in _MMW:
            p16[r0:r0+K_, _PACK16_OFF[k]:_PACK16_OFF[k]+M_] = w[k]
        else:
            p32[r0:r0+K_, _PACK32_OFF[k]:_PACK32_OFF[k]+M_] = w[k]
    z = np.asarray(inputs["z"], np.float32)
    in_maps = []
    for c in range(n_cores):
        zc = z[c*n_loc:(c+1)*n_loc].reshape(n_loc * O, D_IN)
        m = {"wpack16": p16, "wpack32": p32,
             "zT": np.ascontiguousarray(zc.T).astype(np.float16)}
        in_maps.append(m)
    return in_maps


def kernel(**inputs) -> np.ndarray:
    n = inputs["z"].shape[0]
    n_loc = n // N_CORES
    key = ("k", n_loc)
    if key not in _CACHE:
        nc = build_nc(n_loc)
        _CACHE[key] = (nc, _make_runner(nc))
    nc, runner = _CACHE[key]
    res = runner(make_in_maps(inputs, n_loc))
    out = np.concatenate(
        [res[c]["outT"].T.reshape(n_loc, O, CL) for c in range(N_CORES)], axis=0)
    return out
